# revision 1
# baseline (speedup 1.0000x reference)
"""FAGCN propagation kernel for Trainium2 (8 NeuronCores, Bass/Tile).

Math (see reference):
    x1 = x @ w1; x2 = x @ w2                       # [N] gate scalars
    m  = tanh(x1[in_idx] + x2[out_idx]) * adj_vals # [E] edge gates
    out = segment_sum(m[:,None] * x[out_idx], in_idx, N)

Sharding: edges are bucketed by destination node range; core c owns
destination rows [c*N/8, (c+1)*N/8) and computes those output rows.
Within a core, edges are grouped into 128-row destination blocks and
split into chunks of 128 edges. Per chunk, the rows x[src] (bf16, with
x2[src] packed as two extra bf16 columns holding f32 bits) are fetched
by gpsimd.dma_gather (int16 indices, 4 banks of <=32768 rows), which is
rate-limited by Q7 descriptor generation (~9 ns/row, engine-serial) --
the kernel's overall wall. Chunk counts are variable per (block, bank)
(unified to the max over cores for the shared SPMD program).

An optional I-side (K_NI env: trailing blocks gathered via
gpsimd.indirect_dma_start, int32 indices, compact 260B rows in the xei
table) is OFF by default: measured indirect issue cost is ~8.8 ns/row of
the same Q7 engine time plus a slower ring (~20 ns/row), so it cannot
beat dma_gather. NOTE indirect_dma_start is only correct with 2-D output
APs and [128,1] offsets (one index per partition, ISUB=1); multi-column
offset tensors gather wrong rows on HW.

Per chunk (128 edges, lane e = one edge):
  - ACT: T[e, r] = tanh(x1_block[r] + x2[src_e])  (x1 row broadcast via
    a K=1 PE matmul into PSUM, x2 as per-partition bias)
  - DVE: SM0[e, r] = (iota_r == dst_local_e) * adj_e
         SM[e, r] = SM0[e, r] * T[e, r]
  - PE : psum[r, f] += SM.T @ Xg   (accumulate over the block's chunks)
Block writeback: PSUM -> SBUF; one DMA per group of G blocks.
"""

import math
import os
from contextlib import ExitStack
from dataclasses import dataclass

import ml_dtypes
import numpy as np

import concourse.bass as bass
import concourse.bacc as bacc
import concourse.tile as tile
import concourse.mybir as mybir
from concourse import bass_utils

F32 = mybir.dt.float32
BF16 = mybir.dt.bfloat16
I32 = mybir.dt.int32
I16 = mybir.dt.int16
AF = mybir.ActivationFunctionType
OP = mybir.AluOpType

NP_BF16 = ml_dtypes.bfloat16

RW = 256  # bf16 elements per xe row (512B): 128 x cols + 2 gate cols + pad
RWI = 130  # bf16 elements per compact (indirect-side) row (260B)


@dataclass(frozen=True)
class Cfg:
    n_nodes: int
    n_cores: int
    g: int  # destination blocks per gather/writeback group
    ni: int  # number of blocks (at the end) gathered via indirect DMA

    @property
    def npc(self) -> int:
        return self.n_nodes // self.n_cores

    @property
    def rb(self) -> int:
        return math.ceil(self.npc / 128)

    @property
    def rows_pc(self) -> int:
        return self.rb * 128

    @property
    def npad(self) -> int:
        return math.ceil(self.n_nodes / 128) * 128

    @property
    def nbank(self) -> int:
        return max(1, math.ceil(self.npad / 32768))

    @property
    def brows(self) -> int:
        return math.ceil(self.npad / self.nbank / 128) * 128

    @property
    def na(self) -> int:
        return self.rb - self.ni


@dataclass
class Plan:
    """Static per-core schedule (counts only; shared by build + host_prep)."""

    # chA[c][gi][beta] = chunk count for A-group gi, bank beta
    chA: list  # per core
    # chI[c][gi] = chunk count for I-group gi
    chI: list
    # chunks per block, in consumption order: chB[c][b] = list of
    # (side, col) where col indexes the group's xg tile (A: per-bank
    # local col encoded as (beta, col)).
    chB: list
    wa: list  # [NBANK] max A chunks per (group, bank)
    wi: int  # max I chunks per group
    mw: int  # max total chunks per group (meta width)
    # tailA[gi][beta]: max over cores of used idx count in the LAST chunk
    # of (group gi, bank beta); the gather can stop there (num_idxs_reg)
    tailA: list = None


def make_plan(in_idx, out_idx, cfg: Cfg):
    """Compute the static chunk schedule for each core."""
    in_idx = np.asarray(in_idx)
    out_idx = np.asarray(out_idx)
    NPC, RB, NBANK, BROWS, G = cfg.npc, cfg.rb, cfg.nbank, cfg.brows, cfg.g
    NA = cfg.na
    assert RB % G == 0 and NA % G == 0
    chA, chI, chB = [], [], []
    wa = [0] * NBANK
    wi = 0
    mw = 0
    for c in range(cfg.n_cores):
        base = c * NPC
        sel = (in_idx >= base) & (in_idx < base + NPC)
        dstg = (in_idx[sel] - base).astype(np.int64)
        src = out_idx[sel].astype(np.int64)
        blk = dstg >> 7
        bank = src // BROWS
        cntAB = np.bincount(blk * NBANK + bank, minlength=RB * NBANK).reshape(
            RB, NBANK
        )
        cntB = cntAB.sum(axis=1)
        a_ch = np.maximum(1, np.ceil(cntAB / 128).astype(int))  # [RB, NBANK]
        i_ch = np.maximum(1, np.ceil(cntB / 128).astype(int))  # [RB]
        coreA, coreI, coreB = [], [], []
        for g in range(RB // G):
            blocks = range(g * G, (g + 1) * G)
            if g * G < NA:
                gch = [int(a_ch[list(blocks), b].sum()) for b in range(NBANK)]
                coreA.append(gch)
                for b in range(NBANK):
                    wa[b] = max(wa[b], gch[b])
                mw = max(mw, sum(gch))
                off = [0] * NBANK
                for b0 in blocks:
                    lst = []
                    for beta in range(NBANK):
                        for _ in range(a_ch[b0, beta]):
                            lst.append((beta, off[beta]))
                            off[beta] += 1
                    coreB.append(lst)
            else:
                gch = int(i_ch[list(blocks)].sum())
                coreI.append(gch)
                wi = max(wi, gch)
                mw = max(mw, gch)
                off = 0
                for b0 in blocks:
                    lst = []
                    for _ in range(i_ch[b0]):
                        lst.append((-1, off))
                        off += 1
                    coreB.append(lst)
        chA.append(coreA)
        chI.append(coreI)
        chB.append(coreB)
    return Plan(chA=chA, chI=chI, chB=chB, wa=wa, wi=wi, mw=mw)


def _indirect_q(nc, out_ap, in_ap, idx_ap, queue: str):
    inst = nc.gpsimd.indirect_dma_start(
        out=out_ap,
        out_offset=None,
        in_=in_ap,
        in_offset=bass.IndirectOffsetOnAxis(ap=idx_ap, axis=0),
    )
    inst.ins.queue = queue
    return inst


def build_kernel(cfg: Cfg, plan: Plan):
    nc = bacc.Bacc(
        "TRN2",
        target_bir_lowering=False,
        debug=False,
        num_devices=cfg.n_cores,
    )
    G, RB, NPC = cfg.g, cfg.rb, cfg.npc
    NPAD, NBANK, BROWS = cfg.npad, cfg.nbank, cfg.brows
    NG = RB // G
    NGA = cfg.na // G
    XPAD = NBANK * BROWS
    WA, WI, MW = plan.wa, plan.wi, plan.mw
    # the plan is per-core but instruction schedule must be identical on
    # all cores (SPMD): use per-core values only for data, per-core MAX
    # for the schedule. We instead build ONE program using core 0's
    # schedule -- so the plan must be made identical across cores by
    # construction (host pads counts to the max over cores).
    chA, chI, chB = plan.chA[0], plan.chI[0], plan.chB[0]

    xe_h = nc.dram_tensor("xe", [XPAD, RW], BF16, kind="ExternalInput")
    xei_h = (
        nc.dram_tensor("xei", [NPAD, RWI], BF16, kind="ExternalInput")
        if NG > NGA
        else None
    )
    w12_h = nc.dram_tensor("w12", [128, 2], F32, kind="ExternalInput")
    xts_h = nc.dram_tensor("xts", [128, NPC], F32, kind="ExternalInput")
    # A-side int16 gather indices, wrap-16 layout, per (group, bank):
    # flat columns dimension: sum over A-groups of wa[beta] (fixed width)
    bidx_h = (
        nc.dram_tensor(
            "bidx", [NBANK, NGA, 128, max(WA) * 8], I16, kind="ExternalInput"
        )
        if NGA
        else None
    )
    # I-side int32 indices [128, (NG-NGA)*WI]
    iidx_h = (
        nc.dram_tensor("iidx", [128, (NG - NGA) * WI], I32, kind="ExternalInput")
        if NG > NGA
        else None
    )
    # meta per group: dst_local (f32 bits) cols [0:MW], adj cols [MW:2MW]
    meta_h = nc.dram_tensor("meta", [NG, 128, 2 * MW], I32, kind="ExternalInput")
    out_h = nc.dram_tensor("out", [cfg.rows_pc, 128], F32, kind="ExternalOutput")

    s1_own_h = nc.dram_tensor("s1_own", [cfg.rows_pc, 1], F32, kind="Internal")
    s2_own_h = nc.dram_tensor("s2_own", [NPC, 1], F32, kind="Internal")
    s2_all_h = nc.dram_tensor("s2_all", [NPAD, 1], F32, kind="Internal")

    xe = xe_h.ap()
    out = out_h.ap()
    groups = [list(range(cfg.n_cores))]

    with tile.TileContext(nc) as tc, ExitStack() as ctx:
        singles = ctx.enter_context(tc.tile_pool(name="singles", bufs=1))
        xtp = ctx.enter_context(tc.tile_pool(name="xtp", bufs=2))
        gpool = ctx.enter_context(tc.tile_pool(name="gather", bufs=2))
        ipool = ctx.enter_context(tc.tile_pool(name="idx", bufs=2))
        mpool = ctx.enter_context(tc.tile_pool(name="meta", bufs=2))
        bpool = ctx.enter_context(tc.tile_pool(name="blk", bufs=3))
        tpool = ctx.enter_context(tc.tile_pool(name="tfield", bufs=3))
        smpool = ctx.enter_context(tc.tile_pool(name="sm", bufs=4))
        opool = ctx.enter_context(tc.tile_pool(name="osb", bufs=2))
        pspool = ctx.enter_context(tc.tile_pool(name="ps", bufs=2, space="PSUM"))
        psb = ctx.enter_context(tc.tile_pool(name="psb", bufs=2, space="PSUM"))
        ps12pool = ctx.enter_context(tc.tile_pool(name="ps12", bufs=2, space="PSUM"))

        # ---- constants ----
        iota_i = singles.tile([128, 128], I32)
        nc.gpsimd.iota(iota_i[:], pattern=[[1, 128]], base=0, channel_multiplier=0)
        iota_m = singles.tile([128, 128], BF16)
        nc.vector.tensor_copy(iota_m[:], iota_i[:])
        ones_t = singles.tile([1, 128], F32)
        nc.vector.memset(ones_t[:], 1.0)

        w12_sb = singles.tile([128, 2], F32)
        nc.sync.dma_start(w12_sb[:], w12_h.ap())

        # ---- gate scalars: s12_own = x_slice @ [w1 w2] ----
        s12_sb = singles.tile([2, NPC], F32)
        XTW = 3328
        for t0 in range(0, NPC, XTW):
            w0 = min(XTW, NPC - t0)
            xt_t = xtp.tile([128, XTW], F32, tag="xt")
            nc.sync.dma_start(xt_t[:, :w0], xts_h.ap()[:, t0 : t0 + w0])
            for t1 in range(0, w0, 128):
                w1 = min(128, w0 - t1)
                ps12 = ps12pool.tile([2, 128], F32, tag="ps12")
                nc.tensor.matmul(
                    ps12[:, :w1],
                    lhsT=w12_sb[:],
                    rhs=xt_t[:, t1 : t1 + w1],
                    start=True,
                    stop=True,
                )
                nc.vector.tensor_copy(
                    s12_sb[:, t0 + t1 : t0 + t1 + w1], ps12[:, :w1]
                )

        nc.sync.dma_start(s1_own_h.ap()[0:NPC, :], s12_sb[0:1, :])
        nc.sync.dma_start(s2_own_h.ap(), s12_sb[1:2, :])
        if cfg.rows_pc > NPC:
            zt = singles.tile([1, cfg.rows_pc - NPC], F32)
            nc.vector.memset(zt[:], 0.0)
            nc.sync.dma_start(s1_own_h.ap()[NPC : cfg.rows_pc, :], zt[:])

        # ---- allgather x2, pack into xe gate columns ----
        nc.gpsimd.collective_compute(
            "AllGather",
            OP.bypass,
            groups,
            ins=[s2_own_h.ap()],
            outs=[s2_all_h.ap()[0 : cfg.n_nodes, :]],
        )
        tc.strict_bb_all_engine_barrier()

        s2cols = NPAD // 128
        s2sb = singles.tile([128, s2cols], F32)
        s2_src = s2_all_h.ap().rearrange("(p c) x -> p (c x)", p=128)
        gate_dsts = [xe[:NPAD, 128:130].bitcast(F32)]
        if xei_h is not None:
            gate_dsts.append(xei_h.ap()[:, 128:130].bitcast(F32))
        CSP = 256
        for c0 in range(0, s2cols, CSP):
            c1 = min(c0 + CSP, s2cols)
            nc.sync.dma_start(s2sb[:, c0:c1], s2_src[:, c0:c1])
            for gd in gate_dsts:
                gcol = gd.rearrange("(p c) x -> p (c x)", p=128)
                nc.sync.dma_start(gcol[:, c0:c1], s2sb[:, c0:c1])
        tc.strict_bb_all_engine_barrier()

        # ---- main loop ----
        iq = 0
        for g in range(NG):
            is_a = g < NGA
            xgb = []
            if is_a:
                for beta in range(NBANK):
                    nch = chA[g][beta]
                    n_idx = nch * 128
                    bidx_t = ipool.tile([128, max(WA) * 8], I16, tag=f"bidx{beta}")
                    nc.sync.dma_start(bidx_t[:], bidx_h.ap()[beta, g, :, :])
                    xg = gpool.tile([128, WA[beta], RW], BF16, tag=f"xg{beta}")
                    nc.gpsimd.dma_gather(
                        out_ap=xg[:, 0:nch, :],
                        in_ap=xe[beta * BROWS : (beta + 1) * BROWS, :],
                        idxs_ap=bidx_t[:],
                        num_idxs=n_idx,
                        num_idxs_reg=n_idx,
                        elem_size=RW,
                        single_packet=False,
                    )
                    xgb.append(xg)
            else:
                gi = g - NGA
                nch = chI[gi]
                idx_t = ipool.tile([128, WI], I32, tag="iidx")
                nc.sync.dma_start(
                    idx_t[:, 0:nch], iidx_h.ap()[:, gi * WI : gi * WI + nch]
                )
                xg = gpool.tile([128, WI, RWI], BF16, tag="xgi")
                ISUB = 1  # one chunk per indirect DMA: [128,1] offsets is
                # the only layout the HW ucode handles correctly
                for c0 in range(0, nch, ISUB):
                    cc = min(ISUB, nch - c0)
                    _indirect_q(
                        nc,
                        xg[:, c0 : c0 + cc, :],
                        xei_h.ap(),
                        idx_t[:, c0 : c0 + cc],
                        "qPoolDynamic",
                    )
                xgb.append(xg)

            meta_t = mpool.tile([128, 2 * MW], I32, tag="meta")
            nc.sync.dma_start(meta_t[:], meta_h.ap()[g, :, :])
            s1r_t = bpool.tile([1, G * 128], F32, tag="s1r")
            nc.sync.dma_start(
                s1r_t[:], s1_own_h.ap()[g * G * 128 : (g + 1) * G * 128, :]
            )

            osb = opool.tile([128, G, 128], F32, tag="osb")
            for bi in range(G):
                b = g * G + bi
                bps = psb.tile([128, 128], F32, tag="bps")
                nc.tensor.matmul(
                    bps[:],
                    lhsT=ones_t[:],
                    rhs=s1r_t[:, bi * 128 : (bi + 1) * 128],
                    start=True,
                    stop=True,
                )
                ps = pspool.tile([128, 128], F32, tag="acc")
                cols = chB[b]
                # meta column offset for this block within the group
                moff = sum(len(chB[g * G + j]) for j in range(bi))
                for k, (beta, ci) in enumerate(cols):
                    xg = xgb[beta] if beta >= 0 else xgb[0]
                    mcol = moff + k
                    s2col = xg[:, ci, 128:130].bitcast(F32)
                    tt = tpool.tile([128, 128], BF16, tag="tt")
                    nc.scalar.activation(
                        tt[:], bps[:], AF.Tanh, bias=s2col, scale=1.0
                    )
                    sm0 = smpool.tile([128, 128], BF16, tag="sm0")
                    nc.vector.tensor_scalar(
                        out=sm0[:],
                        in0=iota_m[:],
                        scalar1=meta_t[:, mcol : mcol + 1].bitcast(F32),
                        scalar2=meta_t[:, MW + mcol : MW + mcol + 1].bitcast(F32),
                        op0=OP.is_equal,
                        op1=OP.mult,
                    )
                    sm = smpool.tile([128, 128], BF16, tag="sm")
                    nc.vector.tensor_tensor(
                        out=sm[:], in0=sm0[:], in1=tt[:], op=OP.mult
                    )
                    nc.tensor.matmul(
                        ps[:],
                        lhsT=sm[:],
                        rhs=xg[:, ci, 0:128],
                        start=(k == 0),
                        stop=(k == len(cols) - 1),
                    )
                nc.vector.tensor_copy(osb[:, bi, :], ps[:])
            dst = out[g * G * 128 : (g + 1) * G * 128, :].rearrange(
                "(bi p) f -> p bi f", p=128
            )
            nc.sync.dma_start(dst, osb[:, :, :])

    nc.compile()
    return nc


def host_prep(x, w1, w2, adj_vals, in_idx, out_idx, cfg: Cfg, plan: Plan):
    N = cfg.n_nodes
    NPC, RB = cfg.npc, cfg.rb
    NBANK, BROWS, G = cfg.nbank, cfg.brows, cfg.g
    NG = RB // G
    NGA = cfg.na // G
    WA, WI, MW = plan.wa, plan.wi, plan.mw
    chA, chI, chB = plan.chA[0], plan.chI[0], plan.chB[0]

    x = np.asarray(x, np.float32)
    xe = np.zeros((NBANK * BROWS, RW), NP_BF16)
    xe[:N, :128] = x.astype(NP_BF16)
    xei = None
    if cfg.ni > 0:
        xei = np.zeros((cfg.npad, RWI), NP_BF16)
        xei[:N, :128] = xe[:N, :128]
    w12 = np.ascontiguousarray(
        np.stack([np.asarray(w1, np.float32), np.asarray(w2, np.float32)], axis=1)
    )

    in_idx = np.asarray(in_idx)
    out_idx = np.asarray(out_idx)
    adj_vals = np.asarray(adj_vals, np.float32)

    in_maps = []
    for c in range(cfg.n_cores):
        base = c * NPC
        sel = (in_idx >= base) & (in_idx < base + NPC)
        src = out_idx[sel].astype(np.int64)
        dstg = (in_idx[sel] - base).astype(np.int64)
        av = adj_vals[sel]
        blk = dstg >> 7
        bank = np.where(blk < cfg.na, src // BROWS, 0)
        order = np.lexsort((bank, blk))
        src, dstg, av, blk, bank = (
            src[order],
            dstg[order],
            av[order],
            blk[order],
            bank[order],
        )
        key = blk * NBANK + bank
        cnt = np.bincount(key, minlength=RB * NBANK).reshape(RB, NBANK)
        starts = np.concatenate([[0], np.cumsum(cnt.ravel())[:-1]]).reshape(
            RB, NBANK
        )

        bidx = (
            np.zeros((NBANK, NGA, 128, max(WA) * 8), np.int16) if NGA else None
        )
        iidx = (
            np.zeros((128, (NG - NGA) * WI), np.int32) if NG > NGA else None
        )
        meta = np.zeros((NG, 128, 2 * MW), np.int32)

        for g in range(NG):
            blocks = list(range(g * G, (g + 1) * G))
            mcol = 0
            if g < NGA:
                # per bank: concat blocks' slots (each padded to chunks)
                for beta in range(NBANK):
                    col = 0
                    idx_flat = np.zeros((chA[g][beta] * 128,), np.int16)
                    for b0 in blocks:
                        nch = sum(1 for (bb, _) in chB[b0] if bb == beta)
                        s, n = starts[b0, beta], cnt[b0, beta]
                        seg = np.zeros((nch * 128,), np.int16)
                        seg[:n] = (src[s : s + n] - beta * BROWS).astype(np.int16)
                        idx_flat[col * 128 : (col + nch) * 128] = seg
                        col += nch
                    # wrap-16, replicate x8
                    wrap = idx_flat.reshape(-1, 16).T  # [16, n/16]
                    bidx[beta, g, :, : wrap.shape[1]] = np.tile(wrap, (8, 1))
            else:
                gi = g - NGA
                col = 0
                for b0 in blocks:
                    nch = len(chB[b0])
                    s, n = starts[b0, 0], cnt[b0, :].sum()
                    seg = np.zeros((nch * 128,), np.int32)
                    seg[:n] = src[s : s + n].astype(np.int32)
                    # column-major into [128, cols]: slot j -> [j%128, j//128]
                    iidx[:, gi * WI + col : gi * WI + col + nch] = seg.reshape(
                        nch, 128
                    ).T
                    col += nch
            # meta (both sides): per block, per chunk column
            for bi, b0 in enumerate(blocks):
                cols_b = chB[b0]
                if g < NGA:
                    # chunk order: bank-major as in chB
                    doff = {beta: starts[b0, beta] for beta in range(NBANK)}
                    dcnt = {beta: cnt[b0, beta] for beta in range(NBANK)}
                    used = {beta: 0 for beta in range(NBANK)}
                    for k, (beta, _ci) in enumerate(cols_b):
                        s = doff[beta] + used[beta] * 128
                        n = min(128, dcnt[beta] - used[beta] * 128)
                        n = max(0, n)
                        dl = np.zeros((128,), np.float32)
                        aa = np.zeros((128,), np.float32)
                        if n > 0:
                            dl[:n] = (dstg[s : s + n] - b0 * 128).astype(
                                np.float32
                            )
                            aa[:n] = av[s : s + n]
                        meta[g, :, mcol] = dl.view(np.int32)
                        meta[g, :, MW + mcol] = aa.view(np.int32)
                        used[beta] += 1
                        mcol += 1
                else:
                    s0 = starts[b0, 0]
                    ntot = cnt[b0, :].sum()
                    for k in range(len(cols_b)):
                        s = s0 + k * 128
                        n = max(0, min(128, ntot - k * 128))
                        dl = np.zeros((128,), np.float32)
                        aa = np.zeros((128,), np.float32)
                        if n > 0:
                            dl[:n] = (dstg[s : s + n] - b0 * 128).astype(
                                np.float32
                            )
                            aa[:n] = av[s : s + n]
                        meta[g, :, mcol] = dl.view(np.int32)
                        meta[g, :, MW + mcol] = aa.view(np.int32)
                        mcol += 1

        xts = np.ascontiguousarray(x[base : base + NPC].T)
        m = {"xe": xe, "w12": w12, "xts": xts, "meta": meta}
        if xei is not None:
            m["xei"] = xei
        if bidx is not None:
            m["bidx"] = bidx
        if iidx is not None:
            m["iidx"] = iidx
        in_maps.append(m)
    return in_maps


def _unify_plans(plan: Plan, cfg: Cfg) -> Plan:
    """SPMD needs one schedule: pad all cores' chunk counts to the max.

    We rebuild chB for core 0 as-if its per-(block,bank) counts were the
    per-position maxima, and overwrite all cores' entries with it."""
    n_cores = len(plan.chA)
    NBANK, G = cfg.nbank, cfg.g
    NGA = cfg.na // G
    NG = cfg.rb // G
    # max per (block, bank) over cores, derived back from chB lists
    nblk = cfg.rb
    cnts = np.zeros((n_cores, nblk, NBANK + 1), int)
    for c in range(n_cores):
        for b in range(nblk):
            for beta, _ in plan.chB[c][b]:
                cnts[c, b, beta if beta >= 0 else NBANK] += 1
    mx = cnts.max(axis=0)  # [nblk, NBANK+1]
    chA, chI, chB = [], [], []
    tailA = []
    wa = [0] * NBANK
    wi = 0
    mw = 0
    for g in range(NG):
        blocks = list(range(g * G, (g + 1) * G))
        if g < NGA:
            gch = [int(mx[blocks, beta].sum()) for beta in range(NBANK)]
            chA.append(gch)
            tailA.append([0] * NBANK)
            for beta in range(NBANK):
                wa[beta] = max(wa[beta], gch[beta])
            mw = max(mw, sum(gch))
            off = [0] * NBANK
            for b0 in blocks:
                lst = []
                for beta in range(NBANK):
                    for _ in range(mx[b0, beta]):
                        lst.append((beta, off[beta]))
                        off[beta] += 1
                chB.append(lst)
        else:
            gch = int(mx[blocks, NBANK].sum())
            chI.append(gch)
            wi = max(wi, gch)
            mw = max(mw, gch)
            off = 0
            for b0 in blocks:
                lst = []
                for _ in range(mx[b0, NBANK]):
                    lst.append((-1, off))
                    off += 1
                chB.append(lst)
    return Plan(
        chA=[chA] * n_cores,
        chI=[chI] * n_cores,
        chB=[chB] * n_cores,
        wa=wa,
        wi=wi,
        mw=mw,
        tailA=tailA,
    )


_NC_CACHE: dict = {}


def run(x, w1, w2, adj_vals, in_idx, out_idx, trace=False, **kw):
    N = int(np.asarray(x).shape[0])
    n_cores = 8
    g = int(os.environ.get("K_G", "2"))
    ni_blocks = int(os.environ.get("K_NI", "0"))
    cfg = Cfg(n_nodes=N, n_cores=n_cores, g=g, ni=ni_blocks)
    plan0 = make_plan(in_idx, out_idx, cfg)
    plan = _unify_plans(plan0, cfg)
    key = (
        cfg,
        tuple(tuple(x_) for x_ in plan.chA[0]),
        tuple(plan.chI[0]),
        tuple(len(b) for b in plan.chB[0]),
    )
    if key not in _NC_CACHE:
        _NC_CACHE[key] = build_kernel(cfg, plan)
    nc = _NC_CACHE[key]
    in_maps = host_prep(x, w1, w2, adj_vals, in_idx, out_idx, cfg, plan)
    res = bass_utils.run_bass_kernel_spmd(
        nc, in_maps, core_ids=list(range(n_cores)), trace=trace, **kw
    )
    parts = [res.results[c]["out"][: cfg.npc] for c in range(n_cores)]
    outv = np.ascontiguousarray(np.concatenate(parts, axis=0), dtype=np.float32)
    return outv, res


def kernel(x, w1, w2, adj_vals, in_idx, out_idx):
    outv, _ = run(x, w1, w2, adj_vals, in_idx, out_idx)
    return outv



# revision 5
# speedup vs baseline: 1.4141x; 1.4141x over previous
"""FAGCN propagation kernel for Trainium2 (8 NeuronCores, Bass/Tile). v2

Math (see reference):
    x1 = x @ w1; x2 = x @ w2                       # [N] gate scalars
    m  = tanh(x1[in_idx] + x2[out_idx]) * adj_vals # [E] edge gates
    out = segment_sum(m[:,None] * x[out_idx], in_idx, N)

Sharding: edges bucketed by destination; core c owns dst rows
[c*N/8, (c+1)*N/8) (12544 padded rows = 98 blocks of 128). Blocks are
processed in groups of G; within a (group, bank) segment, edges are
sorted by dst and packed contiguously into 128-edge chunks (chunks may
straddle block boundaries -> one matmul per (chunk, block) pair, with
the one-hot masking foreign lanes to zero).

Per chunk, x[src] rows (bf16, 256B) are fetched by gpsimd.dma_gather
(int16 indices, 4 banks of <=32768 rows). Q7 descriptor generation
(~7.4 ns/row + ~1us/instr, engine-serial) is the wall. Padding slots
use index 0 (NOT -1: the ucode trims trailing negatives but the decode
stage reserves ring space from num_idxs_reg, so trimming desyncs the
descriptor-ring bookkeeping and wedges the device).

Per (chunk, block) pair (lane e = one edge):
  - DVE: x2g[e] = sum_f Xg[e,f]*w2[f]      (tensor_tensor_reduce, 1/chunk)
  - ACT: T[e,r] = tanh(x1_row[r] + x2g[e]) (x1 row broadcast via K=1 PE
         matmul into PSUM, x2g as per-partition bias)
  - DVE: SM[e,r] = SM0[e,r] * T[e,r]       (SM0 = host-streamed bf16
         one-hot*adj: adj_e at column dst_local_e, zeros elsewhere)
  - PE : psum[r,f] += SM.T @ Xg            (accumulate over block pairs)
Block writeback: PSUM -> SBUF; one DMA per group.

The one-hot*adj matrices (SM0) are pure index/adj preprocessing built on
host and streamed densely from HBM (~66MB/core) -- this replaced a
pathological 1145ns/chunk DVE tensor_scalar (is_equal+mult with two
per-partition SBUF scalar operands) that was the old bottleneck. x2 is
computed on-device from the gathered rows, which drops the gate columns
from the gather (512B->256B rows) and eliminates the old allgather +
strided gate-packing preamble (~400us).
"""

import math
import os
from contextlib import ExitStack
from dataclasses import dataclass

import ml_dtypes
import numpy as np

import concourse.bass as bass
import concourse.bacc as bacc
import concourse.tile as tile
import concourse.mybir as mybir
from concourse import bass_utils

F32 = mybir.dt.float32
BF16 = mybir.dt.bfloat16
I32 = mybir.dt.int32
I16 = mybir.dt.int16
AF = mybir.ActivationFunctionType
OP = mybir.AluOpType

NP_BF16 = ml_dtypes.bfloat16

N_NODES = 100000
N_CORES = 8
HID = 128
NPC = N_NODES // N_CORES  # 12500
RB = math.ceil(NPC / 128)  # 98
RPC = RB * 128  # 12544
NPAD = math.ceil(N_NODES / 128) * 128  # 100096
NBANK = 4
BROWS = math.ceil(NPAD / NBANK / 128) * 128  # 25088
XPAD = NBANK * BROWS  # 100352


@dataclass
class Plan:
    g: int
    ng: int
    nch: list  # [NG][NBANK] unified chunk counts (max over cores)
    wa: list  # [NBANK] max nch over groups
    pairs: list  # [NG] list of (b_local, beta, ci), b-major issue order
    npg: list  # [NG] pairs per group
    maxpg: int
    goff: list  # [NG] pair-offset prefix sums
    totp: int
    x2off: list  # [NG][NBANK] chunk column offset within group
    maxch: int  # max chunks per group

    def key(self):
        return (
            self.g,
            tuple(tuple(r) for r in self.nch),
            tuple(tuple(p) for p in (tuple(x) for x in self.pairs)),
        )


def _core_edges(in_idx, out_idx, adj_vals, c, G):
    """Sorted (dstg, src, adj, g, beta, rank-in-segment) for core c."""
    NG = RB // G
    base = c * NPC
    sel = (in_idx >= base) & (in_idx < base + NPC)
    dstg = (in_idx[sel] - base).astype(np.int64)
    src = out_idx[sel].astype(np.int64)
    adj = adj_vals[sel] if adj_vals is not None else None
    g = dstg // (G * 128)
    beta = src // BROWS
    order = np.lexsort((dstg, beta, g))
    dstg, src, g, beta = dstg[order], src[order], g[order], beta[order]
    if adj is not None:
        adj = adj[order]
    key = g * NBANK + beta
    cnt = np.bincount(key, minlength=NG * NBANK).astype(np.int64)
    seg_start = np.concatenate([[0], np.cumsum(cnt)])[:-1]
    rank = np.arange(len(dstg)) - seg_start[key]
    return dstg, src, adj, g, beta, rank, cnt.reshape(NG, NBANK)


def make_plan(in_idx, out_idx, G):
    NG = RB // G
    in_idx = np.asarray(in_idx)
    out_idx = np.asarray(out_idx)
    cnt = np.zeros((N_CORES, NG, NBANK), np.int64)
    pair_rows = []
    for c in range(N_CORES):
        dstg, src, _, g, beta, rank, cnt_c = _core_edges(
            in_idx, out_idx, None, c, G
        )
        cnt[c] = cnt_c
        ci = rank // 128
        blocal = (dstg >> 7) - g * G
        pair_rows.append(
            np.unique(np.stack([g, beta, blocal, ci], 1), axis=0)
        )
    allpairs = np.unique(np.concatenate(pair_rows, 0), axis=0)
    nch = np.maximum(1, np.ceil(cnt / 128)).astype(np.int64).max(axis=0)
    wa = [int(nch[:, b].max()) for b in range(NBANK)]
    pairs, npg, goff, x2off = [], [], [], []
    off = 0
    maxch = 0
    for g_ in range(NG):
        rows = allpairs[allpairs[:, 0] == g_]
        # b-major issue order: (blocal, beta, ci)
        lst = sorted((int(b), int(be), int(c_)) for _, be, b, c_ in rows)
        pairs.append(lst)
        npg.append(len(lst))
        goff.append(off)
        off += len(lst)
        xo = [0] * NBANK
        s = 0
        for b_ in range(NBANK):
            xo[b_] = s
            s += int(nch[g_, b_])
        x2off.append(xo)
        maxch = max(maxch, s)
    return Plan(
        g=G,
        ng=NG,
        nch=[[int(x) for x in row] for row in nch],
        wa=wa,
        pairs=pairs,
        npg=npg,
        maxpg=max(npg),
        goff=goff,
        totp=off,
        x2off=x2off,
        maxch=maxch,
    )


def build_kernel(plan: Plan):
    nc = bacc.Bacc(
        "TRN2",
        target_bir_lowering=False,
        debug=False,
        num_devices=N_CORES,
    )
    G, NG = plan.g, plan.ng
    WAmax = max(plan.wa)
    QN = int(os.environ.get("K_QN", "0"))
    SP = bool(int(os.environ.get("K_SP", "0")))

    xe_h = nc.dram_tensor("xe", [XPAD, HID], BF16, kind="ExternalInput")
    xts_h = nc.dram_tensor("xts", [128, NPC], F32, kind="ExternalInput")
    w1_h = nc.dram_tensor("w1c", [128, 1], F32, kind="ExternalInput")
    w2r_h = nc.dram_tensor("w2row", [128, 128], BF16, kind="ExternalInput")
    bidx_h = nc.dram_tensor(
        "bidx", [NBANK, NG, 128, WAmax * 8], I16, kind="ExternalInput"
    )
    sm0_h = nc.dram_tensor(
        "sm0", [128, plan.totp * 128], BF16, kind="ExternalInput"
    )
    out_h = nc.dram_tensor("out", [RPC, 128], F32, kind="ExternalOutput")

    xe = xe_h.ap()
    out = out_h.ap()

    with tile.TileContext(nc) as tc, ExitStack() as ctx:
        singles = ctx.enter_context(tc.tile_pool(name="singles", bufs=1))
        xtp = ctx.enter_context(tc.tile_pool(name="xtp", bufs=2))
        ipool = ctx.enter_context(tc.tile_pool(name="idx", bufs=2))
        gpool = ctx.enter_context(tc.tile_pool(name="gather", bufs=2))
        spool = ctx.enter_context(tc.tile_pool(name="sm0s", bufs=2))
        x2pool = ctx.enter_context(tc.tile_pool(name="x2", bufs=2))
        scrp = ctx.enter_context(tc.tile_pool(name="scr", bufs=2))
        tpool = ctx.enter_context(tc.tile_pool(name="tt", bufs=3))
        smpool = ctx.enter_context(tc.tile_pool(name="sm", bufs=3))
        opool = ctx.enter_context(tc.tile_pool(name="osb", bufs=2))
        ps12p = ctx.enter_context(tc.tile_pool(name="ps12", bufs=2, space="PSUM"))
        bpsp = ctx.enter_context(tc.tile_pool(name="bps", bufs=2, space="PSUM"))
        pspool = ctx.enter_context(tc.tile_pool(name="acc", bufs=2, space="PSUM"))

        ones_t = singles.tile([1, 128], BF16)
        nc.vector.memset(ones_t[:], 1.0)
        w1_sb = singles.tile([128, 1], F32)
        nc.sync.dma_start(w1_sb[:], w1_h.ap())
        w2r_sb = singles.tile([128, 128], BF16)
        nc.sync.dma_start(w2r_sb[:], w2r_h.ap())

        # ---- gate row: s1row[0, r] = x1 of own dst row r (bf16) ----
        s1row = singles.tile([1, RPC], BF16)
        nc.vector.memset(s1row[:], 0.0)
        XTW = 1664
        for t0 in range(0, NPC, XTW):
            w0 = min(XTW, NPC - t0)
            xt_t = xtp.tile([128, XTW], F32, tag="xt")
            nc.sync.dma_start(xt_t[:, :w0], xts_h.ap()[:, t0 : t0 + w0])
            for t1 in range(0, w0, 128):
                ww = min(128, w0 - t1)
                ps12 = ps12p.tile([1, 128], F32, tag="ps12")
                nc.tensor.matmul(
                    ps12[:, :ww],
                    lhsT=w1_sb[:],
                    rhs=xt_t[:, t1 : t1 + ww],
                    start=True,
                    stop=True,
                )
                nc.vector.tensor_copy(
                    s1row[:, t0 + t1 : t0 + t1 + ww], ps12[:, :ww]
                )

        # warm memset: never-gathered lanes must hold finite values
        for _rep in range(2):
            for b_ in range(NBANK):
                xg = gpool.tile([128, plan.wa[b_], HID], BF16, tag=f"xg{b_}")
                nc.vector.memset(xg[:], 0.0)

        # ---- main loop ----
        for g_ in range(NG):
            xgb = []
            for b_ in range(NBANK):
                nch = plan.nch[g_][b_]
                bt = ipool.tile([128, WAmax * 8], I16, tag=f"bidx{b_}")
                nc.sync.dma_start(
                    bt[:, : nch * 8], bidx_h.ap()[b_, g_, :, : nch * 8]
                )
                xg = gpool.tile([128, plan.wa[b_], HID], BF16, tag=f"xg{b_}")
                nc.gpsimd.dma_gather(
                    out_ap=xg[:, 0:nch, :],
                    in_ap=xe[b_ * BROWS : (b_ + 1) * BROWS, :],
                    idxs_ap=bt[:, : nch * 8],
                    num_idxs=nch * 128,
                    num_idxs_reg=nch * 128,
                    elem_size=HID,
                    single_packet=SP,
                    queue_num=(b_ if QN else 0),
                )
                xgb.append(xg)

            npg = plan.npg[g_]
            smt = spool.tile([128, plan.maxpg * 128], BF16, tag="sm0")
            nc.sync.dma_start(
                smt[:, : npg * 128],
                sm0_h.ap()[
                    :, plan.goff[g_] * 128 : (plan.goff[g_] + npg) * 128
                ],
            )

            # x2g per chunk = sum_f Xg[:,f]*w2[f]. NOTE: tensor_tensor_reduce
            # would fuse these two ops but hangs real HW (passes CoreSim) --
            # use separate tensor_tensor + tensor_reduce.
            x2t = x2pool.tile([128, plan.maxch], F32, tag="x2")
            for b_ in range(NBANK):
                for ci in range(plan.nch[g_][b_]):
                    kk = plan.x2off[g_][b_] + ci
                    scr = scrp.tile([128, HID], BF16, tag="scr")
                    nc.vector.tensor_tensor(
                        out=scr[:],
                        in0=xgb[b_][:, ci, :],
                        in1=w2r_sb[:],
                        op=OP.mult,
                    )
                    nc.vector.tensor_reduce(
                        out=x2t[:, kk : kk + 1],
                        in_=scr[:],
                        axis=mybir.AxisListType.X,
                        op=OP.add,
                    )

            osb = opool.tile([128, G, 128], F32, tag="osb")
            plist = plan.pairs[g_]
            for bi in range(G):
                prs = [
                    (p_i, beta, ci)
                    for p_i, (bb, beta, ci) in enumerate(plist)
                    if bb == bi
                ]
                b = g_ * G + bi
                bps = bpsp.tile([128, 128], F32, tag="bps")
                nc.tensor.matmul(
                    bps[:],
                    lhsT=ones_t[:],
                    rhs=s1row[:, b * 128 : (b + 1) * 128],
                    start=True,
                    stop=True,
                )
                ps = pspool.tile([128, 128], F32, tag="acc")
                for j, (p_i, beta, ci) in enumerate(prs):
                    kk = plan.x2off[g_][beta] + ci
                    tt = tpool.tile([128, 128], BF16, tag="tt")
                    nc.scalar.activation(
                        tt[:], bps[:], AF.Tanh, bias=x2t[:, kk : kk + 1],
                        scale=1.0,
                    )
                    sm = smpool.tile([128, 128], BF16, tag="sm")
                    nc.vector.tensor_tensor(
                        out=sm[:],
                        in0=smt[:, p_i * 128 : (p_i + 1) * 128],
                        in1=tt[:],
                        op=OP.mult,
                    )
                    nc.tensor.matmul(
                        ps[:],
                        lhsT=sm[:],
                        rhs=xgb[beta][:, ci, :],
                        start=(j == 0),
                        stop=(j == len(prs) - 1),
                    )
                nc.vector.tensor_copy(osb[:, bi, :], ps[:])
            dst = out[g_ * G * 128 : (g_ + 1) * G * 128, :].rearrange(
                "(bi p) f -> p bi f", p=128
            )
            nc.sync.dma_start(dst, osb[:, :, :])

    nc.compile()
    return nc


def host_prep(x, w1, w2, adj_vals, in_idx, out_idx, plan: Plan):
    G, NG = plan.g, plan.ng
    WAmax = max(plan.wa)
    in_idx = np.asarray(in_idx)
    out_idx = np.asarray(out_idx)
    adj_vals = np.asarray(adj_vals, np.float32)
    x = np.asarray(x, np.float32)

    xe = np.zeros((XPAD, HID), NP_BF16)
    xe[:N_NODES] = x.astype(NP_BF16)
    w1c = np.ascontiguousarray(np.asarray(w1, np.float32)[:, None])
    w2row = np.ascontiguousarray(
        np.tile(np.asarray(w2, np.float32).astype(NP_BF16)[None, :], (128, 1))
    )

    # pair position lookup: [NG, G, NBANK, WAmax] -> global pair index
    ppos = np.full((NG, G, NBANK, WAmax), -1, np.int64)
    for g_ in range(NG):
        for p_i, (b_, be, ci) in enumerate(plan.pairs[g_]):
            ppos[g_, b_, be, ci] = plan.goff[g_] + p_i

    in_maps = []
    for c in range(N_CORES):
        dstg, src, adj, g, beta, rank, cnt_c = _core_edges(
            in_idx, out_idx, adj_vals, c, G
        )
        ci = rank // 128
        lane = rank % 128
        blocal = (dstg >> 7) - g * G
        dstl = dstg & 127

        seg_start = np.concatenate([[0], np.cumsum(cnt_c.ravel())])[:-1]
        # Pad with 0 (gathers row 0, masked by SM0), NOT -1: the decode
        # stage reserves ring space from num_idxs_reg while the Q7 ucode
        # trims trailing negatives -> desynced ring bookkeeping -> hang.
        bidx = np.zeros((NBANK, NG, 128, WAmax * 8), np.int16)
        for g_ in range(NG):
            for b_ in range(NBANK):
                n = int(cnt_c[g_, b_])
                nch = plan.nch[g_][b_]
                idx16 = np.zeros((nch * 128,), np.int16)
                s = int(seg_start[g_ * NBANK + b_])
                idx16[:n] = (src[s : s + n] - b_ * BROWS).astype(np.int16)
                wrap = idx16.reshape(-1, 16).T  # [16, nch*8]
                bidx[b_, g_, :, : nch * 8] = np.tile(wrap, (8, 1))

        p_edge = ppos[g, blocal, beta, ci]
        assert (p_edge >= 0).all()
        sm0 = np.zeros((128, plan.totp, 128), NP_BF16)
        sm0[lane, p_edge, dstl] = adj
        sm0 = np.ascontiguousarray(sm0.reshape(128, plan.totp * 128))

        xts = np.ascontiguousarray(x[c * NPC : (c + 1) * NPC].T)
        in_maps.append(
            {
                "xe": xe,
                "xts": xts,
                "w1c": w1c,
                "w2row": w2row,
                "bidx": bidx,
                "sm0": sm0,
            }
        )
    return in_maps


_NC_CACHE: dict = {}


def run(x, w1, w2, adj_vals, in_idx, out_idx, trace=False, **kw):
    G = int(os.environ.get("K_G", "7"))
    plan = make_plan(in_idx, out_idx, G)
    key = plan.key()
    if key not in _NC_CACHE:
        _NC_CACHE[key] = build_kernel(plan)
    nc = _NC_CACHE[key]
    in_maps = host_prep(x, w1, w2, adj_vals, in_idx, out_idx, plan)
    res = bass_utils.run_bass_kernel_spmd(
        nc, in_maps, core_ids=list(range(N_CORES)), trace=trace, **kw
    )
    parts = [res.results[c]["out"][:NPC] for c in range(N_CORES)]
    outv = np.ascontiguousarray(np.concatenate(parts, axis=0), dtype=np.float32)
    return outv, res


def kernel(x, w1, w2, adj_vals, in_idx, out_idx):
    outv, _ = run(x, w1, w2, adj_vals, in_idx, out_idx)
    return outv
